# revision 19
# baseline (speedup 1.0000x reference)
"""BoxRenderLoss Trainium2 kernel (rows-on-partitions layout).

loss = mean over (box, fragment) pairs of masked min-squared-distance between
each box's 10x10 fragment grid and the other box's 100-point sampled boundary,
both directions, / (2*B*FP).

Closed form: the min over the 100 boundary points decomposes into the 4 box
edges; each edge's 25-point uniform grid min is k* = clamp(round(u/s), 0, 24),
val = u - s*k*.  Per (row, i, j):
  dmin = min( ex_i + vqy_j,  ey_j + vqx_i )   where  ex = min(ux^2, vx^2),
  mask = min(mx_i, my_j) < 0                         vq = val^2,
  contribution = dmin * mask                         mx = min(ux, vx)

Everything per-row is 10-wide per coordinate (ux depends only on the x grid
index i), so the heavy chain runs on compact [128, 2*8*10] tiles: partitions
carry 128 row-groups, each partition holds 8 rows (row r = p*8 + s), x|y
side by side.  The (i,j) cross combine uses DVE broadcast access patterns
(step-0 dims) to expand 10-wide arrays to the 100 fragments without
materializing them: [128, 8*10*10] ops.  The mask op runs on GpSimd in
parallel with the DVE adds.  Final reduction: per-partition accum_out from
the masked-multiply, then one K=128 matmul against a ones column collapses
partitions to a single scalar per core; host sums 8 scalars / (2*B*FP).
"""

import os
import numpy as np

# Exact float32 bit patterns of jnp.linspace(0.0, 1.0, 10) (fragment grid).
_LIN10 = np.array(
    [0, 1038323257, 1046711865, 1051372203, 1055100473,
     1057896676, 1059760811, 1061624946, 1063489081, 1065353216],
    dtype=np.uint32,
).view(np.float32)

_B = 4096
_FP = 100
_N_CORES = 8
_BOX_PER_CORE = _B // _N_CORES          # 512
_ROWS = 2 * _BOX_PER_CORE               # 1024 virtual rows per core
_P = 128                                # partitions
_S = _ROWS // _P                        # 8 rows (slots) per partition
_MAGIC = 8388608.0                      # 2^23 round-to-nearest trick

# Input tile column layout (f32 cols):
#   G 0:20 (gx|gy), W 20:36 (w|h), D 36:52 (dx|dy), ONES 52:56,
#   TW 56:72 (tw|th), RI 72:88 (rix|riy), SS 88:104 (sx|sy)
_NCOL = 104
_CG, _CW, _CD, _C1, _CTW, _CRI, _CS = 0, 20, 36, 52, 56, 72, 88

LAST_RESULTS = None  # BassKernelResults of the most recent run (for test.py)

_compiled = {}


def _build_nc():
    import concourse.bass as bass
    import concourse.bacc as bacc
    import concourse.tile as tile
    from concourse import mybir

    f32 = mybir.dt.float32
    bf16 = mybir.dt.bfloat16
    Op = mybir.AluOpType
    Act = mybir.ActivationFunctionType

    nc = bacc.Bacc("TRN2", target_bir_lowering=False, debug=False,
                   num_devices=_N_CORES)
    in_d = nc.dram_tensor("inp", [_P, _NCOL], f32, kind="ExternalInput").ap()
    out_d = nc.dram_tensor("out", [1, 1], f32, kind="ExternalOutput").ap()

    # Chain tiles use (coord, slot, grid) column order -- grid innermost
    # keeps the per-row-scalar broadcast views cheap (fewest dim restarts).
    # Combine operand tiles (EX/VQ/M and friends) use (coord, grid, slot)
    # order -- slot innermost keeps every combine operand's last dim step 1,
    # which the DVE 2x bf16 perf mode requires on all operands.  Producers
    # bridge the two with a transposed output access pattern.
    CSI = [_P, 2, _S, 10]    # chain (coord, slot, grid) view
    XIJS = [_P, 10, 10, _S]  # expanded (i, j, slot) view

    def cs(ap):   # [128, 160] dense -> (c, s, i)
        return ap.rearrange("p (c s i) -> p c s i", c=2, s=_S, i=10)

    def csR(ap):  # (c, s, i)-layout tile read in (c, i, s) iteration order
        return ap.rearrange("p (c s i) -> p c s i", c=2, s=_S, i=10) \
                 .transpose([0, 1, 3, 2])

    def ci(ap):   # (c, i, s)-layout tile, dense view
        return ap.rearrange("p (c i s) -> p c i s", c=2, i=10, s=_S)

    def xi(ap):   # x half [128, 0:80] (i, s) -> broadcast over j
        return (ap[:, 0:80].rearrange("p (i s) -> p i s", i=10)
                .unsqueeze(2).broadcast_to(XIJS))

    def yj(ap):   # y half [128, 80:160] (j, s) -> broadcast over i
        return (ap[:, 80:160].rearrange("p (j s) -> p j s", j=10)
                .unsqueeze(1).broadcast_to(XIJS))

    def xe(ap):   # expanded [128, 800] dense -> (i, j, s)
        return ap.rearrange("p (i j s) -> p i j s", i=10, j=10, s=_S)

    with tile.TileContext(nc) as tc:
        with (
            tc.tile_pool(name="const", bufs=1) as const,
            tc.tile_pool(name="ps", bufs=1, space="PSUM") as ps,
        ):
            IN = const.tile([_P, _NCOL], f32)
            # Two DMAs on separate HWDGE queues (sync + scalar engines).
            # The first carries only G+W -- all the first compute op needs.
            nc.sync.dma_start(IN[:, 0:_CD], in_d[:, 0:_CD])
            nc.scalar.dma_start(IN[:, _CD:_NCOL], in_d[:, _CD:_NCOL])

            # Broadcast views of the per-row inputs.
            G = (IN[:, _CG:_CG + 20].rearrange("p (c i) -> p c i", i=10)
                 .unsqueeze(2).broadcast_to(CSI))
            W = (IN[:, _CW:_CW + 16].rearrange("p (c s) -> p c s", s=_S)
                 .unsqueeze(3).broadcast_to(CSI))
            D = (IN[:, _CD:_CD + 16].rearrange("p (c s) -> p c s", s=_S)
                 .unsqueeze(3).broadcast_to(CSI))
            TW = (IN[:, _CTW:_CTW + 16].rearrange("p (c s) -> p c s", s=_S)
                  .unsqueeze(3).broadcast_to(CSI))
            RI = (IN[:, _CRI:_CRI + 16].rearrange("p (c s) -> p c s", s=_S)
                  .unsqueeze(3).broadcast_to(CSI))
            SS = (IN[:, _CS:_CS + 16].rearrange("p (c s) -> p c s", s=_S)
                  .unsqueeze(3).broadcast_to(CSI))

            U = const.tile([_P, 160], f32)
            V = const.tile([_P, 160], f32)
            T = const.tile([_P, 160], f32)
            K = const.tile([_P, 160], f32)
            SK = const.tile([_P, 160], f32)
            VAL = const.tile([_P, 160], f32)
            M = const.tile([_P, 160], bf16)
            UQ = const.tile([_P, 160], bf16)
            VQ2 = const.tile([_P, 160], bf16)
            EX = const.tile([_P, 160], bf16)
            VQ = const.tile([_P, 160], bf16)
            E1 = const.tile([_P, 800], bf16)
            E2 = const.tile([_P, 800], bf16)
            MM = const.tile([_P, 800], bf16)
            DM = const.tile([_P, 800], bf16)
            SCR = const.tile([_P, 800], bf16)
            part = const.tile([_P, 1], f32)
            outsb = const.tile([1, 1], f32)

            # Compact per-row precompute, x and y merged (FD=160).
            nc.vector.tensor_tensor(cs(T[:]), G, W, Op.mult)       # t0 = g*w
            nc.vector.tensor_tensor(cs(U[:]), cs(T[:]), D, Op.add)  # u
            nc.vector.tensor_tensor(cs(V[:]), TW, cs(U[:]), Op.subtract)
            nc.vector.tensor_tensor(cs(T[:]), cs(U[:]), RI, Op.mult)  # u/s
            nc.scalar.activation(ci(UQ[:]), csR(U[:]), Act.Square)
            nc.vector.tensor_scalar(K[:], T[:], 0.0, _MAGIC, Op.max, Op.add)
            nc.vector.tensor_scalar(K[:], K[:], _MAGIC + 24.0, _MAGIC,
                                    Op.min, Op.subtract)           # k*
            nc.vector.tensor_tensor(cs(SK[:]), cs(K[:]), SS, Op.mult)
            nc.vector.tensor_tensor(VAL[:], U[:], SK[:], Op.subtract)
            nc.vector.tensor_tensor(ci(VQ[:]), csR(VAL[:]), csR(VAL[:]),
                                    Op.mult)                       # val^2
            nc.vector.tensor_tensor(ci(M[:]), csR(U[:]), csR(V[:]), Op.min)
            nc.scalar.activation(ci(VQ2[:]), csR(V[:]), Act.Square)
            nc.vector.tensor_tensor(EX[:], UQ[:], VQ2[:], Op.min)  # ex|ey

            # (i, j) cross combine via broadcast APs (all bf16, slot-inner
            # layout keeps every operand's last dim step 1 -> DVE 2x mode).
            nc.vector.tensor_tensor(xe(E1[:]), xi(EX[:]), yj(VQ[:]), Op.add)
            nc.vector.tensor_tensor(xe(E2[:]), yj(EX[:]), xi(VQ[:]), Op.add)
            nc.vector.tensor_tensor(DM[:], E1[:], E2[:], Op.min)
            nc.vector.tensor_tensor(xe(MM[:]), xi(M[:]), yj(M[:]), Op.min)
            nc.vector.scalar_tensor_tensor(SCR[:], MM[:], 0.0, DM[:],
                                           Op.is_lt, Op.mult,
                                           accum_out=part[:])

            # Partition reduction: [128,1] x ones -> [1,1], then DMA out.
            pr = ps.tile([1, 1], f32)
            nc.tensor.matmul(pr[:], part[:], IN[:, _C1:_C1 + 1])
            nc.vector.tensor_copy(outsb[:], pr[:])
            nc.sync.dma_start(out_d[:], outsb[:])
    nc.compile()
    return nc


def _rows_for_core(boxes_c, targets_c):
    """Per-row input arrays for one core: dict of [1024] f32 arrays."""
    out = {}
    for name in ("w", "d", "tw", "ri", "ss"):
        out[name + "x"] = []
        out[name + "y"] = []
    for A, T in ((boxes_c, targets_c), (targets_c, boxes_c)):
        A = A.astype(np.float32, copy=False)
        T = T.astype(np.float32, copy=False)
        for axis, sfx in ((0, "x"), (1, "y")):
            w = A[:, 2 + axis] - A[:, 0 + axis]
            d = A[:, 0 + axis] - T[:, 0 + axis]
            tw = T[:, 2 + axis] - T[:, 0 + axis]
            with np.errstate(divide="ignore"):
                ri = np.where(tw != 0, np.float32(24.0) / tw, np.float32(0.0))
            out["w" + sfx].append(w)
            out["d" + sfx].append(d)
            out["tw" + sfx].append(tw)
            out["ri" + sfx].append(ri.astype(np.float32))
            out["ss" + sfx].append(tw / np.float32(24.0))
    return {k: np.concatenate(v).astype(np.float32) for k, v in out.items()}


def _input_for_core(boxes_c, targets_c):
    """Build the [128, 104] f32 input tile for one core."""
    r = _rows_for_core(boxes_c, targets_c)
    m = np.zeros((_P, _NCOL), dtype=np.float32)
    m[:, _CG:_CG + 10] = _LIN10
    m[:, _CG + 10:_CG + 20] = _LIN10
    m[:, _C1] = 1.0
    for base, (kx, ky) in (
        (_CW, ("wx", "wy")), (_CD, ("dx", "dy")), (_CTW, ("twx", "twy")),
        (_CRI, ("rix", "riy")), (_CS, ("ssx", "ssy")),
    ):
        m[:, base:base + _S] = r[kx].reshape(_P, _S)
        m[:, base + _S:base + 16] = r[ky].reshape(_P, _S)
    return m


def kernel(boxes: np.ndarray, targets: np.ndarray) -> np.ndarray:
    from concourse.bass_utils import run_bass_kernel_spmd

    global LAST_RESULTS
    boxes = np.ascontiguousarray(boxes, dtype=np.float32)
    targets = np.ascontiguousarray(targets, dtype=np.float32)
    assert boxes.shape == (_B, 4) and targets.shape == (_B, 4)

    if "nc" not in _compiled:
        _compiled["nc"] = _build_nc()
    nc = _compiled["nc"]

    in_maps = []
    for c in range(_N_CORES):
        rows = slice(c * _BOX_PER_CORE, (c + 1) * _BOX_PER_CORE)
        in_maps.append({"inp": _input_for_core(boxes[rows], targets[rows])})

    trace = bool(int(os.environ.get("BOXLOSS_TRACE", "0")))
    res = run_bass_kernel_spmd(nc, in_maps, list(range(_N_CORES)),
                               trace=trace)
    LAST_RESULTS = res

    total = np.float64(0.0)
    for r in res.results:
        total += np.float64(r["out"].reshape(()))
    loss = total / (2.0 * _B * _FP)
    return np.array(loss, dtype=np.float32)
